# revision 5
# baseline (speedup 1.0000x reference)
"""Bass/Tile TRN2 kernel for nn_SRNN: spiking RNN forward + softmax.

Reference semantics (T=128, B=256, D=512, H=1024, O=20):
    w' = w_rec * (1 - I)          # no self-recurrence
    for t in 0..T-2:
        v = ALPHA*v + z @ w'.T + x[t] @ w_in.T - z*THR
        z = (v > THR)
        vo = KAPPA*vo + z @ w_out.T
        out[t+1] = vo
    out[0] = 0
    return softmax(out, axis=2)

Strategy: data-parallel over batch across 8 cores (weights replicated, no
collectives).  All matmuls run in fp16 at 1 cyc/row with *exact-split*
precision: spikes z in {0,1} are exact in fp16, and each weight matrix is
split as w = hi + lo*2^-11 (both fp16); the 2^-11 is carried by a scaled
spike copy (values {0, 2^-11}, exact in fp16), so every product is exact
and only the split residual (~w*2^-22) is lost -> fp32-class accuracy at
2 passes instead of fp32's 4-cycle/row path.

The "- z*THR" term (THR=1) is folded into the weight diagonal: diag(w') = -1.

Per-step layout: state v,z natural [32, H]; matmul = zT-stationary
([128,32] fp16 slices) streaming w'_T tiles [128,512]; psum accumulates
v across 8 k-tiles x 2 passes.  z is transposed each step via 8 PE
transpose-mode ops.  The input projection c = x @ w_in.T is precomputed
for all t with a 3-pass fp16 split matmul and staged in DRAM.
"""

import os

import numpy as np

import concourse.mybir as mybir
import concourse.tile as tile
from concourse import bacc

dt = mybir.dt
F32, F16 = dt.float32, dt.float16
Alu = mybir.AluOpType

T, B, D, H, O = 128, 256, 512, 1024, 20
NCORES = 8
BC = B // NCORES  # 32
THR = 1.0
ALPHA = float(np.exp(-1.0 / 20.0))
KAPPA = float(np.exp(-1.0 / 20.0))
KT = H // 128  # 8 k-tiles over the hidden dim
KD = D // 128  # 4 k-tiles over the input dim
NROW = T * BC  # 4096 rows of (t, b)
NM = NROW // 128  # 32 row-tiles for the input projection
LO_SCALE = 2.0**11
N_STEPS = T - 1  # 127 recurrent steps (t = 0..126)

_IDENT_PAT = [[-1, 128]]


def _load_split(nc, sp, dram, n_slabs, width, name, diag_fill=None):
    """Load [n_slabs*128, width] f32 from DRAM slab-wise; return (hi, los) f16
    tiles [128, n_slabs*width] (slab kk in cols kk*width..).  If diag_fill is
    set, block-diagonal entries of slab kk (cols kk*128..) get that value
    before splitting."""
    hi = sp.tile([128, n_slabs * width], F16, name=f"{name}_hi")
    los = sp.tile([128, n_slabs * width], F16, name=f"{name}_los")
    for kk in range(n_slabs):
        slab = sp.tile([128, width], F32, tag=f"{name}_slab", bufs=2, name=f"{name}_slab")
        nc.sync.dma_start(slab, dram[kk * 128 : (kk + 1) * 128, :])
        if diag_fill is not None:
            blk = slab[:, kk * 128 : (kk + 1) * 128]
            nc.gpsimd.affine_select(
                out=blk, in_=blk, compare_op=Alu.not_equal, fill=diag_fill,
                base=0, pattern=_IDENT_PAT, channel_multiplier=1,
            )
        hs = hi[:, kk * width : (kk + 1) * width]
        nc.vector.tensor_copy(hs, slab)
        dlt = sp.tile([128, width], F32, tag=f"{name}_dlt", bufs=2, name=f"{name}_dlt")
        nc.vector.tensor_tensor(dlt, slab, hs, Alu.subtract)
        nc.vector.tensor_scalar(
            los[:, kk * width : (kk + 1) * width], dlt, LO_SCALE, None, Alu.mult
        )
    return hi, los


def build(n_steps=N_STEPS):
    nc = bacc.Bacc("TRN2", name="srnn")
    x_d = nc.dram_tensor("x", [NROW, D], F32, kind="ExternalInput")
    win_d = nc.dram_tensor("w_in", [H, D], F32, kind="ExternalInput")
    wrec_d = nc.dram_tensor("w_rec", [H, H], F32, kind="ExternalInput")
    wout_d = nc.dram_tensor("w_out", [O, H], F32, kind="ExternalInput")
    out_d = nc.dram_tensor("out", [T, BC, O], F32, kind="ExternalOutput")
    c_d = nc.dram_tensor("c_buf", [NROW, H], F32)

    with tile.TileContext(nc) as tc, tc.tile_pool(name="persist", bufs=1) as pp:
        with (
            tc.tile_pool(name="setup", bufs=1) as sp,
            tc.tile_pool(name="psetup", bufs=2, space="PSUM") as pps,
        ):
            ident = pp.tile([128, 128], F16)
            nc.gpsimd.memset(ident, 0.0)
            nc.gpsimd.affine_select(
                out=ident, in_=ident, compare_op=Alu.not_equal, fill=1.0,
                base=0, pattern=_IDENT_PAT, channel_multiplier=1,
            )

            # ---- w_rec: load, diag=-1 (folds "- z*THR"), fp16 split, transpose ----
            wrh_nat, wrl_nat = _load_split(
                nc, sp, wrec_d[:, :], KT, H, "wrec", diag_fill=-1.0
            )
            # transposed layout: block ki holds w'[ki*128+p, j] for all j
            wT_hi = pp.tile([128, KT * H], F16)
            wT_lo = pp.tile([128, KT * H], F16)
            for src, dst in ((wrh_nat, wT_hi), (wrl_nat, wT_lo)):
                for ki in range(KT):
                    ptr = pps.tile([128, H], F16, tag="ptr")
                    for kj in range(KT):
                        nc.tensor.transpose(
                            ptr[:, kj * 128 : (kj + 1) * 128],
                            src[:, kj * H + ki * 128 : kj * H + (ki + 1) * 128],
                            ident,
                        )
                    nc.vector.tensor_copy(dst[:, ki * H : (ki + 1) * H], ptr)

            # ---- w_in: load, fp16 split, transpose ----
            wih_nat, wil_nat = _load_split(nc, sp, win_d[:, :], KT, D, "win")
            wiT_hi = pp.tile([128, KD * H], F16)
            wiT_lo = pp.tile([128, KD * H], F16)
            for src, dst in ((wih_nat, wiT_hi), (wil_nat, wiT_lo)):
                for kd in range(KD):
                    pti = pps.tile([128, H], F16, tag="pti")
                    for kj in range(KT):
                        nc.tensor.transpose(
                            pti[:, kj * 128 : (kj + 1) * 128],
                            src[:, kj * D + kd * 128 : kj * D + (kd + 1) * 128],
                            ident,
                        )
                    nc.vector.tensor_copy(dst[:, kd * H : (kd + 1) * H], pti)

            # ---- w_out: load, fp16 (hi only; vo does not feed back), transpose ----
            wout_nat = sp.tile([O, H], F32)
            nc.sync.dma_start(wout_nat, wout_d[:, :])
            wout16 = sp.tile([O, H], F16)
            nc.vector.tensor_copy(wout16, wout_nat)
            woT = pp.tile([128, KT * O], F16)
            pto = pps.tile([128, KT * O], F16, tag="pto")
            for ki in range(KT):
                nc.tensor.transpose(
                    pto[:, ki * O : (ki + 1) * O],
                    wout16[:, ki * 128 : (ki + 1) * 128],
                    ident[:O, :O],
                )
            nc.vector.tensor_copy(woT, pto)

            # vo history [32, T*20]; slot 0 stays zero
            vo_hist = pp.tile([BC, T * O], F32)
            nc.vector.memset(vo_hist, 0.0)

        # ---- phase 1: c = x @ w_in.T via 3-pass fp16 split ----
        with (
            tc.tile_pool(name="ph1", bufs=3) as p1,
            tc.tile_pool(name="ph1ps", bufs=2, space="PSUM") as p1ps,
        ):
            for m in range(NM):
                x_nat = p1.tile([128, D], F32, tag="x_nat")
                nc.sync.dma_start(x_nat, x_d[m * 128 : (m + 1) * 128, :])
                xhi_nat = p1.tile([128, D], F16, tag="xhi_nat")
                nc.vector.tensor_copy(xhi_nat, x_nat)
                xlo_nat = p1.tile([128, D], F16, tag="xlo_nat")
                nc.vector.tensor_tensor(xlo_nat, x_nat, xhi_nat, Alu.subtract)
                ptx = p1ps.tile([128, 2 * D], F16, tag="ptx")
                for kd in range(KD):
                    nc.tensor.transpose(
                        ptx[:, kd * 128 : (kd + 1) * 128],
                        xhi_nat[:, kd * 128 : (kd + 1) * 128],
                        ident,
                    )
                    nc.tensor.transpose(
                        ptx[:, D + kd * 128 : D + (kd + 1) * 128],
                        xlo_nat[:, kd * 128 : (kd + 1) * 128],
                        ident,
                    )
                xT = p1.tile([128, 2 * D], F16, tag="xT")  # [hi | lo]
                nc.vector.tensor_copy(xT, ptx)
                xT_his = p1.tile([128, D], F16, tag="xT_his")
                nc.vector.tensor_scalar(xT_his, xT[:, :D], 1.0 / LO_SCALE, None, Alu.mult)

                pc0 = p1ps.tile([128, 512], F32, tag="pc0")
                pc1 = p1ps.tile([128, 512], F32, tag="pc1")
                for nh, pc in ((0, pc0), (1, pc1)):
                    first, last = (0, 0), (KD - 1, 2)
                    for kd in range(KD):
                        pairs = (
                            (xT[:, kd * 128 : (kd + 1) * 128], wiT_hi),
                            (xT_his[:, kd * 128 : (kd + 1) * 128], wiT_lo),
                            (xT[:, D + kd * 128 : D + (kd + 1) * 128], wiT_hi),
                        )
                        for pi, (lhsT, w) in enumerate(pairs):
                            nc.tensor.matmul(
                                pc,
                                lhsT=lhsT,
                                rhs=w[:, kd * H + nh * 512 : kd * H + nh * 512 + 512],
                                start=(kd, pi) == first,
                                stop=(kd, pi) == last,
                            )
                c_stage = p1.tile([128, H], F32, tag="c_stage")
                nc.scalar.copy(c_stage[:, 0:512], pc0)
                nc.scalar.copy(c_stage[:, 512:1024], pc1)
                nc.sync.dma_start(c_d[m * 128 : (m + 1) * 128, :], c_stage)

        # ---- phase 2: recurrent loop ----
        with (
            tc.tile_pool(name="loop", bufs=2) as lp,
            tc.tile_pool(name="cpool", bufs=3) as cp,
            tc.tile_pool(name="lps", bufs=2, space="PSUM") as lps,
        ):
            def make_zT(z):
                """z [32, H] f16 -> (zT [128, KT*32] f16, zTs scaled)."""
                pzt = lps.tile([128, KT * 32], F16, tag="pzt")
                for k in range(KT):
                    nc.tensor.transpose(
                        pzt[:, k * 32 : (k + 1) * 32],
                        z[:, k * 128 : (k + 1) * 128],
                        ident[:32, :32],
                    )
                zT = lp.tile([128, KT * 32], F16, tag="zT")
                nc.vector.tensor_copy(zT, pzt)
                zTs = lp.tile([128, KT * 32], F16, tag="zTs")
                nc.vector.tensor_scalar(zTs, zT, 1.0 / LO_SCALE, None, Alu.mult)
                return zT, zTs

            def vo_mm(zT, t):
                """vo_hist[t] = KAPPA * vo_hist[t-1] + z(t) @ w_out.T"""
                pvo = lps.tile([BC, O], F32, tag="pvo")
                for k in range(KT):
                    nc.tensor.matmul(
                        pvo,
                        lhsT=zT[:, k * 32 : (k + 1) * 32],
                        rhs=woT[:, k * O : (k + 1) * O],
                        start=(k == 0),
                        stop=(k == KT - 1),
                    )
                nc.vector.scalar_tensor_tensor(
                    vo_hist[:, t * O : (t + 1) * O],
                    vo_hist[:, (t - 1) * O : t * O],
                    KAPPA,
                    pvo,
                    Alu.mult,
                    Alu.add,
                )

            # t=0: v(1) = c[0]; z(1) = (v>1); vo(0)=0 (already)
            c_t = cp.tile([BC, H], F32, tag="c_t")
            nc.sync.dma_start(c_t, c_d[0:BC, :])
            v_sb = lp.tile([BC, H], F32, tag="v_sb")
            nc.vector.tensor_copy(v_sb, c_t)
            z = lp.tile([BC, H], F16, tag="z")
            nc.vector.tensor_scalar(z, v_sb, THR, None, Alu.is_gt)
            zT, zTs = make_zT(z)

            for t in range(1, n_steps + 1):
                last = t == n_steps
                vo_mm(zT, t)
                if last:
                    break
                c_t = cp.tile([BC, H], F32, tag="c_t")
                nc.sync.dma_start(c_t, c_d[t * BC : (t + 1) * BC, :])
                u = lp.tile([BC, H], F32, tag="u")
                nc.vector.scalar_tensor_tensor(u, v_sb, ALPHA, c_t, Alu.mult, Alu.add)
                pv0 = lps.tile([BC, 512], F32, tag="pv0")
                pv1 = lps.tile([BC, 512], F32, tag="pv1")
                for nh, pv in ((0, pv0), (1, pv1)):
                    for pi, (zt_op, w) in enumerate(((zT, wT_hi), (zTs, wT_lo))):
                        for k in range(KT):
                            nc.tensor.matmul(
                                pv,
                                lhsT=zt_op[:, k * 32 : (k + 1) * 32],
                                rhs=w[:, k * H + nh * 512 : k * H + nh * 512 + 512],
                                start=(pi == 0 and k == 0),
                                stop=(pi == 1 and k == KT - 1),
                            )
                v_new = lp.tile([BC, H], F32, tag="v_sb")
                z = lp.tile([BC, H], F16, tag="z")
                for nh, pv in ((0, pv0), (1, pv1)):
                    sl = slice(nh * 512, nh * 512 + 512)
                    nc.vector.tensor_tensor(v_new[:, sl], pv, u[:, sl], Alu.add)
                    nc.vector.tensor_scalar(z[:, sl], v_new[:, sl], THR, None, Alu.is_gt)
                v_sb = v_new
                zT, zTs = make_zT(z)

        # ---- softmax over O within each t, and emit ----
        with (
            tc.tile_pool(name="smax", bufs=1) as smp,
        ):
            vo_exp = smp.tile([BC, T * O], F32)
            nc.scalar.activation(vo_exp, vo_hist, mybir.ActivationFunctionType.Exp)
            sums = smp.tile([BC, T], F32)
            nc.vector.tensor_reduce(
                sums,
                vo_exp.rearrange("p (t o) -> p t o", o=O),
                mybir.AxisListType.X,
                Alu.add,
            )
            recip = smp.tile([BC, T], F32)
            nc.vector.reciprocal(recip, sums)
            prob = smp.tile([BC, T * O], F32)
            for o in range(O):
                nc.vector.tensor_tensor(
                    prob.rearrange("p (t o) -> p t o", o=O)[:, :, o],
                    vo_exp.rearrange("p (t o) -> p t o", o=O)[:, :, o],
                    recip,
                    Alu.mult,
                )
            nc.sync.dma_start(
                out_d[:, :, :].rearrange("t b o -> b t o"),
                prob.rearrange("p (t o) -> p t o", o=O),
            )

    nc.compile()
    return nc


_CACHED = {}


def _get_nc(n_steps=N_STEPS):
    if n_steps not in _CACHED:
        _CACHED[n_steps] = build(n_steps)
    return _CACHED[n_steps]


def kernel(x, w_in, w_rec, w_out):
    from concourse.bass_utils import run_bass_kernel_spmd

    nc = _get_nc()
    in_maps = []
    for c in range(NCORES):
        shard = np.ascontiguousarray(x[:, c * BC : (c + 1) * BC, :]).reshape(NROW, D)
        in_maps.append(
            {
                "x": shard.astype(np.float32),
                "w_in": np.ascontiguousarray(w_in, dtype=np.float32),
                "w_rec": np.ascontiguousarray(w_rec, dtype=np.float32),
                "w_out": np.ascontiguousarray(w_out, dtype=np.float32),
            }
        )
    res = run_bass_kernel_spmd(nc, in_maps, core_ids=list(range(NCORES)))
    outs = [r["out"] for r in res.results]  # each [T, BC, O]
    return np.concatenate(outs, axis=1).astype(np.float32)


if __name__ == "__main__":
    rng = np.random.default_rng(0)
    x = rng.standard_normal((T, B, D)).astype(np.float32)
    w_in = (rng.standard_normal((H, D)) * np.sqrt(2.0 / D)).astype(np.float32)
    w_rec = (rng.standard_normal((H, H)) * np.sqrt(2.0 / H)).astype(np.float32)
    w_out = (rng.standard_normal((O, H)) * np.sqrt(2.0 / H)).astype(np.float32)
    out = kernel(x=x, w_in=w_in, w_rec=w_rec, w_out=w_out)
    print(out.shape, out.dtype, out[1, 0, :3])
